# revision 1
# baseline (speedup 1.0000x reference)
"""DeformConv1d (modulated, K=3, stride=1, pad=1, dil=1) on 8 Trainium2
NeuronCores via Bass.

Contract: kernel(**inputs) takes the FULL inputs
  x[16,256,4096] f32, weight[256,256,3] f32, offset[16,3,4096] f32,
  mask[16,3,4096] f32, bias[256] f32
and returns the full output [16,256,4096] f32.

Strategy (data-parallel over batch, 2 batches per core):
  out[b,o,w] = sum_k m[k,w]*(w0*z_k[o,i0] + w1*z_k[o,i0+1]) + bias[o],
  z_k = W_k @ x[b].
  On device, z_k is produced TRANSPOSED ([w,oc] tiles) by matmuls with
  x-slices stationary, staged to DRAM, then two indirect-DMA row gathers
  per tap fetch z_k[i0]/z_k[i0+1]; VectorE applies the interpolation
  weights (precomputed host-side from offset/mask, along with clamped
  indices) and the bias. The output leaves the device transposed and is
  unpermuted on the host.
"""
import numpy as np

import concourse.bass as bass
import concourse.bacc as bacc
import concourse.tile as tile
from concourse import mybir
from concourse.bass_utils import run_bass_kernel_spmd

F32 = mybir.dt.float32
F32R = mybir.dt.float32r
BF16 = mybir.dt.bfloat16
I32 = mybir.dt.int32

B2 = 2          # batches per core
K = 3
W = 4096
NT = W // 128   # 32 w-tiles
N_CORES = 8


def _build(reps: int = 1, fast: bool = False, t1_dve: bool = False, deep: bool = False):
    nc = bacc.Bacc("TRN2", target_bir_lowering=False, debug=False)

    z_dt = F32  # keep staging/gather fp32 for precision; fast only switches matmul to fp32r

    x_in = nc.dram_tensor("x_in", [B2, 128, 2, W], F32, kind="ExternalInput")
    wT_in = nc.dram_tensor("wT_in", [128, K, 2, 256], F32, kind="ExternalInput")
    bias_in = nc.dram_tensor("bias_in", [128, 256], F32, kind="ExternalInput")
    idx0_in = nc.dram_tensor("idx0_in", [B2, 128, K, NT], I32, kind="ExternalInput")
    idx1_in = nc.dram_tensor("idx1_in", [B2, 128, K, NT], I32, kind="ExternalInput")
    c0_in = nc.dram_tensor("c0_in", [B2, 128, K, NT], F32, kind="ExternalInput")
    c1_in = nc.dram_tensor("c1_in", [B2, 128, K, NT], F32, kind="ExternalInput")
    outT = nc.dram_tensor("outT", [B2, 128, NT, 256], F32, kind="ExternalOutput")
    zTs = [[nc.dram_tensor(f"zT_{b}_{k}", [W, 256], z_dt) for k in range(K)]
           for b in range(B2)]

    with tile.TileContext(nc) as tc:
        with (
            tc.tile_pool(name="const", bufs=1) as cpool,
            tc.tile_pool(name="xp", bufs=1) as xpool,
            tc.tile_pool(name="zstage", bufs=8 if deep else 6) as zpool,
            tc.tile_pool(name="gp", bufs=1) as gpool,
            tc.tile_pool(name="coefp", bufs=2) as coefpool,
            tc.tile_pool(name="interp", bufs=2) as ipool,
            tc.tile_pool(name="accp", bufs=1) as apool,
            tc.tile_pool(name="psum", bufs=8, space="PSUM") as psum,
        ):
            w_raw = cpool.tile([128, K, 2, 256], F32, tag="wraw")
            nc.sync.dma_start(out=w_raw[:], in_=wT_in[:])
            if fast:
                w_rt = cpool.tile([128, K, 2, 256], F32R, tag="wr")
                nc.vector.tensor_copy(w_rt[:], w_raw[:])
                w_r = w_rt[:]
            else:
                w_r = w_raw[:]
            bias_sb = cpool.tile([128, 256], F32, tag="bias")
            nc.sync.dma_start(out=bias_sb[:], in_=bias_in[:])
            zero_b = cpool.tile([128, 1], F32, tag="zb")
            nc.gpsimd.memset(zero_b[:], 0.0)

            for rep in range(reps):
                for b in range(B2):
                    x_raw = xpool.tile([128, 2, W], F32, tag="xraw")
                    nc.sync.dma_start(out=x_raw[:], in_=x_in[b])
                    if fast:
                        x_rt = xpool.tile([128, 2, W], F32R, tag="xr")
                        nc.vector.tensor_copy(x_rt[:], x_raw[:])
                        x_r = x_rt[:]
                    else:
                        x_r = x_raw[:]

                    idx0_sb = coefpool.tile([128, K, NT], I32, tag="i0")
                    idx1_sb = coefpool.tile([128, K, NT], I32, tag="i1")
                    c0_sb = coefpool.tile([128, K, NT], F32, tag="c0")
                    c1_sb = coefpool.tile([128, K, NT], F32, tag="c1")
                    nc.sync.dma_start(out=idx0_sb[:], in_=idx0_in[b])
                    nc.sync.dma_start(out=idx1_sb[:], in_=idx1_in[b])
                    nc.sync.dma_start(out=c0_sb[:], in_=c0_in[b])
                    nc.sync.dma_start(out=c1_sb[:], in_=c1_in[b])

                    acc = apool.tile([128, NT, 256], F32, tag="acc")

                    for k in range(K):
                        for wt in range(NT):
                            zp = psum.tile([128, 256], F32, tag="zp")
                            ws = wt * 128
                            for cc in range(2):
                                nc.tensor.matmul(
                                    zp[:],
                                    x_r[:, cc, ws:ws + 128],
                                    w_r[:, k, cc],
                                    start=(cc == 0),
                                    stop=(cc == 1),
                                )
                            zst = zpool.tile([128, 256], z_dt, tag="zst")
                            nc.scalar.activation(
                                zst[:], zp[:],
                                mybir.ActivationFunctionType.Identity,
                                bias=zero_b[:])
                            nc.sync.dma_start(
                                out=zTs[b][k][ws:ws + 128], in_=zst[:])

                        zrows = zTs[b][k][:]  # [4096, 256]
                        H = NT // 2
                        for h in range(2):
                            hs = h * H
                            g0 = gpool.tile([128, H, 256], z_dt, tag="g0")
                            g1 = gpool.tile([128, H, 256], z_dt, tag="g1")
                            for t in range(H):
                                nc.gpsimd.indirect_dma_start(
                                    out=g0[:, t], out_offset=None, in_=zrows,
                                    in_offset=bass.IndirectOffsetOnAxis(
                                        ap=idx0_sb[:, k, hs + t:hs + t + 1],
                                        axis=0))
                                nc.gpsimd.indirect_dma_start(
                                    out=g1[:, t], out_offset=None, in_=zrows,
                                    in_offset=bass.IndirectOffsetOnAxis(
                                        ap=idx1_sb[:, k, hs + t:hs + t + 1],
                                        axis=0))

                            c0b = c0_sb[:, k, hs:hs + H][:, :, None] \
                                .broadcast_to([128, H, 256])
                            c1b = c1_sb[:, k, hs:hs + H][:, :, None] \
                                .broadcast_to([128, H, 256])
                            t0 = ipool.tile([128, H, 256], F32, tag="t0")
                            acch = acc[:, hs:hs + H]
                            nc.vector.tensor_tensor(t0[:], g0[:], c0b,
                                                    mybir.AluOpType.mult)
                            if k == 0:
                                biasb = bias_sb[:][:, None, :].broadcast_to(
                                    [128, H, 256])
                                nc.vector.tensor_tensor(
                                    acch, t0[:], biasb, mybir.AluOpType.add)
                            else:
                                nc.vector.tensor_tensor(
                                    acch, acch, t0[:], mybir.AluOpType.add)
                            t1 = ipool.tile([128, H, 256], F32, tag="t0")
                            eng_t1 = nc.vector if t1_dve else nc.gpsimd
                            eng_t1.tensor_tensor(t1[:], g1[:], c1b,
                                                 mybir.AluOpType.mult)
                            nc.vector.tensor_tensor(
                                acch, acch, t1[:], mybir.AluOpType.add)

                    nc.sync.dma_start(out=outT[b], in_=acc[:])

    nc.compile()
    return nc


def _prep_coeffs(offset, mask):
    """offset/mask [B,K,W] -> idx0,c0,idx1,c1 in [B,128,K,NT] device layout,
    slot (p,t) <-> w = t*128+p. Float op order replicates the reference."""
    B = offset.shape[0]
    base = np.arange(W, dtype=np.float32) * np.float32(1.0) - np.float32(1.0)
    kpos = np.arange(K, dtype=np.float32) * np.float32(1.0)
    bk = (base[None, :] + kpos[:, None]).astype(np.float32)
    p = (bk[None] + offset).astype(np.float32)
    i0f = np.floor(p)
    w1 = (p - i0f).astype(np.float32)
    w0 = (np.float32(1.0) - w1).astype(np.float32)
    i0 = i0f.astype(np.int64)
    i1 = i0 + 1
    v0 = (i0 >= 0) & (i0 < W)
    v1 = (i1 >= 0) & (i1 < W)
    c0 = (mask * w0 * v0).astype(np.float32)
    c1 = (mask * w1 * v1).astype(np.float32)
    idx0 = np.clip(i0, 0, W - 1).astype(np.int32)
    idx1 = np.clip(i1, 0, W - 1).astype(np.int32)

    def lay(a):
        return np.ascontiguousarray(a.reshape(B, K, NT, 128).transpose(0, 3, 1, 2))

    return lay(idx0), lay(c0), lay(idx1), lay(c1)


def _core_inputs(x, weight, offset, mask, bias, core):
    b0 = 2 * core
    idx0, c0, idx1, c1 = _prep_coeffs(offset[b0:b0 + 2], mask[b0:b0 + 2])
    OC = weight.shape[0]
    return {
        "x_in": np.ascontiguousarray(
            x[b0:b0 + 2].reshape(2, 2, 128, W).transpose(0, 2, 1, 3)
        ).astype(np.float32),
        "wT_in": np.ascontiguousarray(
            weight.transpose(2, 1, 0).reshape(K, 2, 128, OC)
            .transpose(2, 0, 1, 3)).astype(np.float32),
        "bias_in": np.ascontiguousarray(
            np.broadcast_to(bias.reshape(1, -1), (128, OC))).astype(np.float32),
        "idx0_in": idx0, "idx1_in": idx1, "c0_in": c0, "c1_in": c1,
    }


_NC_CACHE = {}


def _get_nc(reps=1, fast=False):
    key = (reps, fast)
    if key not in _NC_CACHE:
        _NC_CACHE[key] = _build(reps=reps, fast=fast)
    return _NC_CACHE[key]


_DISPATCH = None


def _get_dispatch(nc):
    """Build (once) a cached jitted shard_map dispatcher over 8 cores,
    mirroring bass2jax.run_bass_via_pjrt but without per-call retracing."""
    global _DISPATCH
    if _DISPATCH is not None:
        return _DISPATCH
    import jax
    from jax.sharding import Mesh, PartitionSpec
    from jax.experimental.shard_map import shard_map
    from concourse import bass2jax, mybir as mb
    bass2jax.install_neuronx_cc_hook()

    partition_name = (nc.partition_id_tensor.name
                      if nc.partition_id_tensor else None)
    in_names, out_names, out_avals, zero_outs = [], [], [], []
    for alloc in nc.m.functions[0].allocations:
        if not isinstance(alloc, mb.MemoryLocationSet):
            continue
        name = alloc.memorylocations[0].name
        if alloc.kind == "ExternalInput":
            if name != partition_name:
                in_names.append(name)
        elif alloc.kind == "ExternalOutput":
            shape = tuple(alloc.tensor_shape)
            dtype = mb.dt.np(alloc.dtype)
            out_names.append(name)
            out_avals.append(jax.core.ShapedArray(shape, dtype))
            zero_outs.append(np.zeros(shape, dtype))
    n_params = len(in_names)
    n_outs = len(out_avals)
    all_in_names = list(in_names) + list(out_names)
    if partition_name is not None:
        all_in_names.append(partition_name)

    def _body(*args):
        operands = list(args)
        if partition_name is not None:
            operands.append(bass2jax.partition_id_tensor())
        outs = bass2jax._bass_exec_p.bind(
            *operands,
            out_avals=tuple(out_avals),
            in_names=tuple(all_in_names),
            out_names=tuple(out_names),
            lowering_input_output_aliases=(),
            sim_require_finite=True,
            sim_require_nnan=True,
            nc=nc,
        )
        return tuple(outs)

    devices = jax.devices()[:N_CORES]
    mesh = Mesh(np.asarray(devices), ("core",))
    in_specs = (PartitionSpec("core"),) * (n_params + n_outs)
    out_specs = (PartitionSpec("core"),) * n_outs
    donate = tuple(range(n_params, n_params + n_outs))
    sharded = jax.jit(
        shard_map(_body, mesh=mesh, in_specs=in_specs, out_specs=out_specs,
                  check_rep=False),
        donate_argnums=donate, keep_unused=True)
    _DISPATCH = (sharded, in_names, out_names, out_avals, zero_outs)
    return _DISPATCH


def kernel(x, weight, offset, mask, bias):
    x = np.asarray(x, dtype=np.float32)
    weight = np.asarray(weight, dtype=np.float32)
    offset = np.asarray(offset, dtype=np.float32)
    mask = np.asarray(mask, dtype=np.float32)
    bias = np.asarray(bias, dtype=np.float32)

    nc = _get_nc(fast=True)
    sharded, in_names, out_names, out_avals, zero_outs = _get_dispatch(nc)
    ins_list = [_core_inputs(x, weight, offset, mask, bias, core)
                for core in range(N_CORES)]
    concat_in = [np.concatenate([ins_list[c][n] for c in range(N_CORES)],
                                axis=0) for n in in_names]
    concat_zeros = [np.zeros((N_CORES * z.shape[0], *z.shape[1:]), z.dtype)
                    for z in zero_outs]
    out_arrs = sharded(*concat_in, *concat_zeros)
    i = out_names.index("outT")
    allT = np.asarray(out_arrs[i]).reshape(N_CORES, *out_avals[i].shape)

    out = np.empty((16, 256, W), np.float32)
    for core in range(N_CORES):
        out[2 * core:2 * core + 2] = np.ascontiguousarray(
            allT[core].transpose(0, 3, 2, 1).reshape(2, 256, W))
    return out



# revision 4
# speedup vs baseline: 16.8915x; 16.8915x over previous
"""DeformConv1d (modulated, K=3, stride=1, pad=1, dil=1) on 8 Trainium2
NeuronCores via Bass.

Contract: kernel(**inputs) takes the FULL inputs
  x[16,256,4096] f32, weight[256,256,3] f32, offset[16,3,4096] f32,
  mask[16,3,4096] f32, bias[256] f32
and returns the full output [16,256,4096] f32.

Strategy (data-parallel over batch, 2 batches per core), v2 (fp16):
  out[b,o,w] = sum_k c0[k,w]*z_k[o,i0] + c1[k,w]*z_k[o,i0+1] + bias[o],
  z_k = W_k @ x[b],  c0 = mask*(1-frac)*valid0, c1 = mask*frac*valid1.

  On device (all fp16 except PSUM):
  - z_k produced transposed ([w,oc] tiles) by fp16 matmuls; PSUM chunks
    [128,1024]f32 evacuated by ScalarE to fp16 SBUF; ONE big DMA per
    (b,k) stages z_k to DRAM.
  - ONE batched indirect row-gather per (b,k,tap) fetches all 4096
    sampled rows ([128,32] indices -> [128,32,256] fp16).
  - VectorE interp: in-place g *= coef (coef uploaded as duplicated
    pairs so the broadcast keeps innermost stride 1 => DVE 2x mode),
    then acc += g; bias folded into the k==0 add.
  - Output leaves transposed fp16; host unpermutes/casts.
"""
import numpy as np

import concourse.bass as bass
import concourse.bacc as bacc
import concourse.tile as tile
from concourse import mybir
from concourse.bass_utils import run_bass_kernel_spmd

F32 = mybir.dt.float32
F16 = mybir.dt.float16
I32 = mybir.dt.int32

B2 = 2          # batches per core
K = 3
W = 4096
NT = W // 128   # 32 w-tiles
OC = 256
N_CORES = 8


def _build(reps: int = 1):
    nc = bacc.Bacc("TRN2", target_bir_lowering=False, debug=False)

    x_in = nc.dram_tensor("x_in", [B2, 128, 2, W], F16, kind="ExternalInput")
    w_in = nc.dram_tensor("w_in", [128, K, 2, OC], F16, kind="ExternalInput")
    bias_in = nc.dram_tensor("bias_in", [128, 128, 2], F16, kind="ExternalInput")
    idx_in = nc.dram_tensor("idx_in", [B2, 128, K, 2, NT], I32,
                            kind="ExternalInput")
    coef_in = nc.dram_tensor("coef_in", [B2, 128, K, 2, NT, 2], F16,
                             kind="ExternalInput")
    outT = nc.dram_tensor("outT", [B2, 128, NT, OC], F16, kind="ExternalOutput")
    zTs = [[nc.dram_tensor(f"zT_{b}_{k}", [NT, 128, OC], F16)
            for k in range(K)] for b in range(B2)]

    with tile.TileContext(nc) as tc:
        with (
            tc.tile_pool(name="const", bufs=1) as cpool,
            tc.tile_pool(name="xp", bufs=2) as xpool,
            tc.tile_pool(name="cf", bufs=2) as cfpool,
            tc.tile_pool(name="zstage", bufs=2) as zpool,
            tc.tile_pool(name="gp", bufs=2) as gpool,
            tc.tile_pool(name="accp", bufs=2) as apool,
            tc.tile_pool(name="psum", bufs=3, space="PSUM") as psum,
        ):
            w_sb = cpool.tile([128, K, 2, OC], F16, tag="w")
            nc.sync.dma_start(out=w_sb[:], in_=w_in[:])
            bias_sb = cpool.tile([128, 128, 2], F16, tag="bias")
            nc.sync.dma_start(out=bias_sb[:], in_=bias_in[:])
            biasv = bias_sb[:][:, None, :, :].broadcast_to([128, NT, 128, 2])

            for rep in range(reps):
                for b in range(B2):
                    x_sb = xpool.tile([128, 2, W], F16, tag="x")
                    nc.sync.dma_start(out=x_sb[:], in_=x_in[b])
                    idx_sb = cfpool.tile([128, K, 2, NT], I32, tag="idx")
                    nc.sync.dma_start(out=idx_sb[:], in_=idx_in[b])
                    coef_sb = cfpool.tile([128, K, 2, NT, 2], F16, tag="coef")
                    nc.sync.dma_start(out=coef_sb[:], in_=coef_in[b])

                    acc = apool.tile([128, NT, 128, 2], F16, tag="acc")

                    for k in range(K):
                        zst = zpool.tile([128, NT, OC], F16, tag="zst")
                        for c8 in range(8):  # 512 output positions per chunk
                            zp = psum.tile([128, 4, OC], F32, tag="zp")
                            for q in range(4):
                                ws = c8 * 512 + q * 128
                                for cc in range(2):
                                    nc.tensor.matmul(
                                        zp[:, q],
                                        x_sb[:, cc, ws:ws + 128],
                                        w_sb[:, k, cc],
                                        start=(cc == 0),
                                        stop=(cc == 1),
                                    )
                            nc.scalar.activation(
                                zst[:, c8 * 4:(c8 + 1) * 4], zp[:],
                                mybir.ActivationFunctionType.Copy)
                        # one staging write for the whole (b,k): SBUF
                        # [128,NT,256] -> DRAM rows [w=t*128+p, oc]
                        nc.sync.dma_start(
                            out=zTs[b][k][:].transpose([1, 0, 2]), in_=zst[:])

                        zrows = zTs[b][k][:].flatten_outer_dims()  # [4096,256]
                        g0 = gpool.tile([128, NT, OC], F16, tag="g0")
                        g1 = gpool.tile([128, NT, OC], F16, tag="g1")
                        # NOTE: batching many indices into one indirect DMA
                        # ([128,NT] offsets) computes wrong source addresses
                        # on HW (works in the interpreter) — keep [128,1].
                        for t in range(NT):
                            nc.gpsimd.indirect_dma_start(
                                out=g0[:, t], out_offset=None, in_=zrows,
                                in_offset=bass.IndirectOffsetOnAxis(
                                    ap=idx_sb[:, k, 0, t:t + 1], axis=0))
                            nc.gpsimd.indirect_dma_start(
                                out=g1[:, t], out_offset=None, in_=zrows,
                                in_offset=bass.IndirectOffsetOnAxis(
                                    ap=idx_sb[:, k, 1, t:t + 1], axis=0))

                        g0v = g0[:].rearrange("p t (j l) -> p t j l", l=2)
                        g1v = g1[:].rearrange("p t (j l) -> p t j l", l=2)
                        c0v = coef_sb[:, k, 0][:, :, None, :].broadcast_to(
                            [128, NT, 128, 2])
                        c1v = coef_sb[:, k, 1][:, :, None, :].broadcast_to(
                            [128, NT, 128, 2])
                        nc.vector.tensor_tensor(g0v, g0v, c0v,
                                                mybir.AluOpType.mult)
                        if k == 0:
                            nc.vector.tensor_tensor(acc[:], g0v, biasv,
                                                    mybir.AluOpType.add)
                        else:
                            nc.vector.tensor_tensor(acc[:], acc[:], g0v,
                                                    mybir.AluOpType.add)
                        nc.vector.tensor_tensor(g1v, g1v, c1v,
                                                mybir.AluOpType.mult)
                        nc.vector.tensor_tensor(acc[:], acc[:], g1v,
                                                mybir.AluOpType.add)

                    nc.sync.dma_start(
                        out=outT[b],
                        in_=acc[:].rearrange("p t j l -> p t (j l)"))

    nc.compile()
    return nc


def _prep_coeffs(offset, mask):
    """offset/mask [B,K,W] -> idx [B,128,K,2,NT] i32 and coef
    [B,128,K,2,NT,2] f16 in device layout, slot (p,t) <-> w = t*128+p."""
    B = offset.shape[0]
    base = np.arange(W, dtype=np.float32) - np.float32(1.0)
    kpos = np.arange(K, dtype=np.float32)
    p = (base[None, :] + kpos[:, None])[None] + offset
    i0f = np.floor(p)
    w1 = (p - i0f).astype(np.float32)
    w0 = np.float32(1.0) - w1
    i0 = i0f.astype(np.int64)
    i1 = i0 + 1
    c0 = (mask * w0 * ((i0 >= 0) & (i0 < W))).astype(np.float16)
    c1 = (mask * w1 * ((i1 >= 0) & (i1 < W))).astype(np.float16)
    idx0 = np.clip(i0, 0, W - 1).astype(np.int32)
    idx1 = np.clip(i1, 0, W - 1).astype(np.int32)

    def lay(a):  # [B,K,W] -> [B,128,K,NT]
        return a.reshape(B, K, NT, 128).transpose(0, 3, 1, 2)

    idx = np.ascontiguousarray(
        np.stack([lay(idx0), lay(idx1)], axis=3))      # [B,128,K,2,NT]
    coef = np.stack([lay(c0), lay(c1)], axis=3)        # [B,128,K,2,NT]
    coef = np.ascontiguousarray(
        np.repeat(coef[..., None], 2, axis=-1))        # [B,128,K,2,NT,2]
    return idx, coef


def _core_inputs(x, weight, offset, mask, bias, core):
    b0 = 2 * core
    idx, coef = _prep_coeffs(offset[b0:b0 + 2], mask[b0:b0 + 2])
    return {
        "x_in": np.ascontiguousarray(
            x[b0:b0 + 2].reshape(2, 2, 128, W).transpose(0, 2, 1, 3)
        ).astype(np.float16),
        "w_in": np.ascontiguousarray(
            weight.transpose(2, 1, 0).reshape(K, 2, 128, OC)
            .transpose(2, 0, 1, 3)).astype(np.float16),
        "bias_in": np.ascontiguousarray(
            np.broadcast_to(bias.reshape(1, 128, 2), (128, 128, 2))
        ).astype(np.float16),
        "idx_in": idx, "coef_in": coef,
    }


_NC_CACHE = {}


def _get_nc(reps=1):
    if reps not in _NC_CACHE:
        _NC_CACHE[reps] = _build(reps=reps)
    return _NC_CACHE[reps]


_DISPATCH = {}


def _get_dispatch(nc, key=0):
    """Build (once) a cached jitted shard_map dispatcher over 8 cores,
    mirroring bass2jax.run_bass_via_pjrt but without per-call retracing."""
    if key in _DISPATCH:
        return _DISPATCH[key]
    import jax
    from jax.sharding import Mesh, PartitionSpec
    from jax.experimental.shard_map import shard_map
    from concourse import bass2jax, mybir as mb
    bass2jax.install_neuronx_cc_hook()

    partition_name = (nc.partition_id_tensor.name
                      if nc.partition_id_tensor else None)
    in_names, out_names, out_avals, zero_outs = [], [], [], []
    for alloc in nc.m.functions[0].allocations:
        if not isinstance(alloc, mb.MemoryLocationSet):
            continue
        name = alloc.memorylocations[0].name
        if alloc.kind == "ExternalInput":
            if name != partition_name:
                in_names.append(name)
        elif alloc.kind == "ExternalOutput":
            shape = tuple(alloc.tensor_shape)
            dtype = mb.dt.np(alloc.dtype)
            out_names.append(name)
            out_avals.append(jax.core.ShapedArray(shape, dtype))
            zero_outs.append(np.zeros(shape, dtype))
    n_params = len(in_names)
    n_outs = len(out_avals)
    all_in_names = list(in_names) + list(out_names)
    if partition_name is not None:
        all_in_names.append(partition_name)

    def _body(*args):
        operands = list(args)
        if partition_name is not None:
            operands.append(bass2jax.partition_id_tensor())
        outs = bass2jax._bass_exec_p.bind(
            *operands,
            out_avals=tuple(out_avals),
            in_names=tuple(all_in_names),
            out_names=tuple(out_names),
            lowering_input_output_aliases=(),
            sim_require_finite=True,
            sim_require_nnan=True,
            nc=nc,
        )
        return tuple(outs)

    devices = jax.devices()[:N_CORES]
    mesh = Mesh(np.asarray(devices), ("core",))
    in_specs = (PartitionSpec("core"),) * (n_params + n_outs)
    out_specs = (PartitionSpec("core"),) * n_outs
    donate = tuple(range(n_params, n_params + n_outs))
    sharded = jax.jit(
        shard_map(_body, mesh=mesh, in_specs=in_specs, out_specs=out_specs,
                  check_rep=False),
        donate_argnums=donate, keep_unused=True)
    _DISPATCH[key] = (sharded, in_names, out_names, out_avals, zero_outs)
    return _DISPATCH[key]


def kernel(x, weight, offset, mask, bias):
    x = np.asarray(x, dtype=np.float32)
    weight = np.asarray(weight, dtype=np.float32)
    offset = np.asarray(offset, dtype=np.float32)
    mask = np.asarray(mask, dtype=np.float32)
    bias = np.asarray(bias, dtype=np.float32)

    nc = _get_nc(reps=1)
    sharded, in_names, out_names, out_avals, zero_outs = _get_dispatch(nc)
    ins_list = [_core_inputs(x, weight, offset, mask, bias, core)
                for core in range(N_CORES)]
    concat_in = [np.concatenate([ins_list[c][n] for c in range(N_CORES)],
                                axis=0) for n in in_names]
    concat_zeros = [np.zeros((N_CORES * z.shape[0], *z.shape[1:]), z.dtype)
                    for z in zero_outs]
    out_arrs = sharded(*concat_in, *concat_zeros)
    i = out_names.index("outT")
    allT = np.asarray(out_arrs[i]).reshape(N_CORES, *out_avals[i].shape)

    out = np.empty((16, OC, W), np.float32)
    for core in range(N_CORES):
        # allT[core]: [2, 128, NT, OC] (p, t, oc) -> [2, OC, W=t*128+p]
        out[2 * core:2 * core + 2] = (
            allT[core].astype(np.float32).transpose(0, 3, 2, 1)
            .reshape(2, OC, W))
    return out


# revision 8
# speedup vs baseline: 204.8415x; 12.1269x over previous
"""DeformConv1d (modulated, K=3, stride=1, pad=1, dil=1) on 8 Trainium2
NeuronCores via Bass.

Contract: kernel(**inputs) takes the FULL inputs
  x[16,256,4096] f32, weight[256,256,3] f32, offset[16,3,4096] f32,
  mask[16,3,4096] f32, bias[256] f32
and returns the full output [16,256,4096] f32.

Strategy v4 — block-banded, all-PE (data-parallel over batch, 2/core):
  out[b,:,w] = sum_k sum_w' S_k[w',w] * z_k[w',:] + bias,
  z_k = W_k @ x[b] (transposed on device: z[w',oc]),
  S_k[w',w]  = c0[k,w]*[w'=i0] + c1[k,w]*[w'=i0+1]   (2 nnz per column).

  The sampling offsets are N(0,1), so |i0 - w| <= ~8 << 128: S is block-
  banded with only the (t-1,t),(t,t),(t+1,t) 128x128 blocks nonzero.
  The host builds those dense f16 blocks; the device then needs NO
  gather at all:
    stage 1 (per b,k): z_k into SBUF fp16 via matmul + ScalarE PSUM evac
    stage 2 (per out-tile t): 1 rank-1 bias matmul + up to 9 S-block
      matmuls accumulate in PSUM; VectorE evacuates fp16; DMA out.
  The host asserts the band actually fits in the 3-block window (any
  |i0-w| <= 126 qualifies; valid-sample coefficients make this hold for
  any plausible offset distribution here).
"""
import numpy as np

import concourse.bass as bass
import concourse.bacc as bacc
import concourse.tile as tile
from concourse import mybir
from concourse.bass_utils import run_bass_kernel_spmd

F32 = mybir.dt.float32
F16 = mybir.dt.float16

B2 = 2          # batches per core
K = 3
W = 4096
NT = W // 128   # 32 w-tiles
OC = 256
N_CORES = 8


def _build(reps: int = 1):
    nc = bacc.Bacc("TRN2", target_bir_lowering=False, debug=False)

    x_in = nc.dram_tensor("x_in", [B2, 128, 2, W], F16, kind="ExternalInput")
    w_in = nc.dram_tensor("w_in", [128, K, 2, OC], F16, kind="ExternalInput")
    ones_in = nc.dram_tensor("ones_in", [1, 128], F16, kind="ExternalInput")
    brow_in = nc.dram_tensor("brow_in", [1, OC], F16, kind="ExternalInput")
    s_in = nc.dram_tensor("s_in", [B2, K, 128, NT, 3, 128], F16,
                          kind="ExternalInput")
    outT = nc.dram_tensor("outT", [B2, 128, NT, OC], F16,
                          kind="ExternalOutput")

    with tile.TileContext(nc) as tc:
        with (
            tc.tile_pool(name="const", bufs=1) as cpool,
            tc.tile_pool(name="xp", bufs=2) as xpool,
            tc.tile_pool(name="sp", bufs=1) as spool,
            tc.tile_pool(name="zp", bufs=1) as zpool,
            tc.tile_pool(name="op", bufs=2) as opool,
            tc.tile_pool(name="zpsum", bufs=2, space="PSUM") as zpsum,
            tc.tile_pool(name="opsum", bufs=2, space="PSUM") as opsum,
        ):
            w_sb = cpool.tile([128, K, 2, OC], F16, tag="w")
            nc.sync.dma_start(out=w_sb[:], in_=w_in[:])
            ones_sb = cpool.tile([1, 128], F16, tag="ones")
            nc.sync.dma_start(out=ones_sb[:], in_=ones_in[:])
            brow_sb = cpool.tile([1, OC], F16, tag="brow")
            nc.sync.dma_start(out=brow_sb[:], in_=brow_in[:])

            for rep in range(reps):
                for b in range(B2):
                    x_sb = xpool.tile([128, 2, W], F16, tag="x")
                    nc.sync.dma_start(out=x_sb[:], in_=x_in[b])
                    s_sb = [spool.tile([128, NT, 3, 128], F16,
                                       tag=f"s{k}", name=f"s_sb{k}")
                            for k in range(K)]
                    for k in range(K):
                        nc.sync.dma_start(out=s_sb[k][:], in_=s_in[b, k])

                    # stage 1: z_k[w',oc] fp16 in SBUF
                    z_sb = [zpool.tile([128, NT, OC], F16,
                                       tag=f"z{k}", name=f"z_sb{k}")
                            for k in range(K)]
                    for k in range(K):
                        for c8 in range(8):
                            zp = zpsum.tile([128, 4, OC], F32, tag="zp")
                            for q in range(4):
                                ws = c8 * 512 + q * 128
                                for cc in range(2):
                                    nc.tensor.matmul(
                                        zp[:, q],
                                        x_sb[:, cc, ws:ws + 128],
                                        w_sb[:, k, cc],
                                        start=(cc == 0),
                                        stop=(cc == 1),
                                    )
                            nc.scalar.activation(
                                z_sb[k][:, c8 * 4:(c8 + 1) * 4], zp[:],
                                mybir.ActivationFunctionType.Copy)

                    # stage 2: banded S^T @ z accumulation per output tile
                    out_sb = opool.tile([128, NT, OC], F16, tag="out")
                    for tq in range(8):
                        op = opsum.tile([128, 4, OC], F32, tag="op")
                        for qq in range(4):
                            t = tq * 4 + qq
                            js = [j for j in (0, 1, 2)
                                  if 0 <= t - 1 + j < NT]
                            nc.tensor.matmul(   # bias: ones^T @ brow
                                op[:, qq], ones_sb[:], brow_sb[:],
                                start=True, stop=False)
                            for ki, k in enumerate(range(K)):
                                for ji, j in enumerate(js):
                                    last = (ki == K - 1 and ji == len(js) - 1)
                                    nc.tensor.matmul(
                                        op[:, qq],
                                        s_sb[k][:, t, j],
                                        z_sb[k][:, t - 1 + j],
                                        start=False, stop=last)
                        nc.vector.tensor_copy(
                            out_sb[:, tq * 4:(tq + 1) * 4], op[:])
                    nc.sync.dma_start(out=outT[b], in_=out_sb[:])

    nc.compile()
    return nc


def _prep_sblocks(offset, mask):
    """offset/mask [B,K,W] -> S blocks [B,K,128,NT,3,128] f16 with
    S[b,k,p',t,j,p] = coef for (w'=(t-1+j)*128+p', w=t*128+p)."""
    B = offset.shape[0]
    base = np.arange(W, dtype=np.float32) - np.float32(1.0)
    kpos = np.arange(K, dtype=np.float32)
    pos = (base[None, :] + kpos[:, None])[None] + offset    # [B,K,W]
    i0f = np.floor(pos)
    w1 = (pos - i0f).astype(np.float32)
    w0 = np.float32(1.0) - w1
    i0 = i0f.astype(np.int64)
    i1 = i0 + 1

    w = np.arange(W, dtype=np.int64)
    p_ = (w % 128)[None, None]
    t_ = (w // 128)[None, None]
    S = np.zeros((B, K, NT, 3, 128, 128), np.float32)
    bb, kk = np.meshgrid(np.arange(B), np.arange(K), indexing="ij")
    bb = bb[..., None] + np.zeros_like(i0)
    kk = kk[..., None] + np.zeros_like(i0)
    for idx, cf in ((i0, mask * w0), (i1, mask * w1)):
        valid = (idx >= 0) & (idx < W) & (cf != 0)
        d = idx - w[None, None]
        assert np.abs(d[valid]).max(initial=0) <= 126, \
            "sampling offsets exceed the 3-block band"
        j = (p_ + d) // 128 + 1
        p2 = (p_ + d) % 128
        tv = (t_ + np.zeros_like(i0))[valid]
        np.add.at(S, (bb[valid], kk[valid], tv, j[valid], p2[valid],
                      (p_ + np.zeros_like(i0))[valid]), cf[valid])
    # [B,K,NT,3,p',p] -> [B,K,p',NT,3,p]
    return np.ascontiguousarray(
        S.transpose(0, 1, 4, 2, 3, 5)).astype(np.float16)


def _core_inputs(x, weight, offset, mask, bias, core):
    b0 = 2 * core
    return {
        "x_in": np.ascontiguousarray(
            x[b0:b0 + 2].reshape(2, 2, 128, W).transpose(0, 2, 1, 3)
        ).astype(np.float16),
        "w_in": np.ascontiguousarray(
            weight.transpose(2, 1, 0).reshape(K, 2, 128, OC)
            .transpose(2, 0, 1, 3)).astype(np.float16),
        "ones_in": np.ones((1, 128), np.float16),
        "brow_in": bias.reshape(1, OC).astype(np.float16),
        "s_in": _prep_sblocks(offset[b0:b0 + 2], mask[b0:b0 + 2]),
    }


_NC_CACHE = {}


def _get_nc(reps=1):
    if reps not in _NC_CACHE:
        _NC_CACHE[reps] = _build(reps=reps)
    return _NC_CACHE[reps]


_DISPATCH = {}


def _get_dispatch(nc, key=0):
    """Build (once) a cached jitted shard_map dispatcher over 8 cores,
    mirroring bass2jax.run_bass_via_pjrt but without per-call retracing."""
    if key in _DISPATCH:
        return _DISPATCH[key]
    import jax
    from jax.sharding import Mesh, PartitionSpec
    from jax.experimental.shard_map import shard_map
    from concourse import bass2jax, mybir as mb
    bass2jax.install_neuronx_cc_hook()

    partition_name = (nc.partition_id_tensor.name
                      if nc.partition_id_tensor else None)
    in_names, out_names, out_avals, zero_outs = [], [], [], []
    for alloc in nc.m.functions[0].allocations:
        if not isinstance(alloc, mb.MemoryLocationSet):
            continue
        name = alloc.memorylocations[0].name
        if alloc.kind == "ExternalInput":
            if name != partition_name:
                in_names.append(name)
        elif alloc.kind == "ExternalOutput":
            shape = tuple(alloc.tensor_shape)
            dtype = mb.dt.np(alloc.dtype)
            out_names.append(name)
            out_avals.append(jax.core.ShapedArray(shape, dtype))
            zero_outs.append(np.zeros(shape, dtype))
    n_params = len(in_names)
    n_outs = len(out_avals)
    all_in_names = list(in_names) + list(out_names)
    if partition_name is not None:
        all_in_names.append(partition_name)

    def _body(*args):
        operands = list(args)
        if partition_name is not None:
            operands.append(bass2jax.partition_id_tensor())
        outs = bass2jax._bass_exec_p.bind(
            *operands,
            out_avals=tuple(out_avals),
            in_names=tuple(all_in_names),
            out_names=tuple(out_names),
            lowering_input_output_aliases=(),
            sim_require_finite=True,
            sim_require_nnan=True,
            nc=nc,
        )
        return tuple(outs)

    devices = jax.devices()[:N_CORES]
    mesh = Mesh(np.asarray(devices), ("core",))
    in_specs = (PartitionSpec("core"),) * (n_params + n_outs)
    out_specs = (PartitionSpec("core"),) * n_outs
    donate = tuple(range(n_params, n_params + n_outs))
    sharded = jax.jit(
        shard_map(_body, mesh=mesh, in_specs=in_specs, out_specs=out_specs,
                  check_rep=False),
        donate_argnums=donate, keep_unused=True)
    _DISPATCH[key] = (sharded, in_names, out_names, out_avals, zero_outs)
    return _DISPATCH[key]


def kernel(x, weight, offset, mask, bias):
    x = np.asarray(x, dtype=np.float32)
    weight = np.asarray(weight, dtype=np.float32)
    offset = np.asarray(offset, dtype=np.float32)
    mask = np.asarray(mask, dtype=np.float32)
    bias = np.asarray(bias, dtype=np.float32)

    nc = _get_nc(reps=1)
    sharded, in_names, out_names, out_avals, zero_outs = _get_dispatch(nc)
    ins_list = [_core_inputs(x, weight, offset, mask, bias, core)
                for core in range(N_CORES)]
    concat_in = [np.concatenate([ins_list[c][n] for c in range(N_CORES)],
                                axis=0) for n in in_names]
    concat_zeros = [np.zeros((N_CORES * z.shape[0], *z.shape[1:]), z.dtype)
                    for z in zero_outs]
    out_arrs = sharded(*concat_in, *concat_zeros)
    i = out_names.index("outT")
    allT = np.asarray(out_arrs[i]).reshape(N_CORES, *out_avals[i].shape)

    out = np.empty((16, OC, W), np.float32)
    for core in range(N_CORES):
        # allT[core]: [2, 128, NT, OC] (p, t, oc) -> [2, OC, W=t*128+p]
        out[2 * core:2 * core + 2] = (
            allT[core].astype(np.float32).transpose(0, 3, 2, 1)
            .reshape(2, OC, W))
    return out
